# revision 2
# baseline (speedup 1.0000x reference)
"""Trainium2 Bass kernel for nn_GAT_n2v_mean (3-layer edge-featured GAT + mean-pool + MLP).

v2 strategy: edges partitioned by dst owner across 8 cores (6250 nodes each,
127-node blocks + trash slot). Per block, ONE batched indirect-DMA gathers all
T*128 src payload rows out of an AllGathered bf16 node table; a second tiny
batched gather fetches per-edge dst attention terms. Segment softmax needs no
max shift (a global shift cancels in the normalization; logits are O(1) for
this input distribution). Attention logits stay fp32 (al_s/al_d ride in the
bf16 table as bit-cast fp32 pairs); values and aggregation matmuls are bf16.
Layer 1 aggregates raw 32-dim inputs and applies W after aggregation.
Per-edge one-hot scatter masks and al_e terms are precomputed on device once
(overlapped with the first AllGather) and DMA'd back per layer. Each
AllGather is split in halves, with next-layer linear tiles interleaved into
the current block loop so the first half overlaps block compute.
"""

import numpy as np
import ml_dtypes

BF16 = ml_dtypes.bfloat16

_PATCHED = False


def _patch_walrus():
    """Enable per-partition vector dynamic offsets in walrus codegen
    (needed for the indirect row gathers; off by default in this path)."""
    global _PATCHED
    if _PATCHED:
        return
    import concourse.bass_utils as _bu
    _orig = _bu.run_command

    def _patched(argv, **kw):
        if any("codegen" in str(a) for a in argv):
            argv = list(argv)
            i = argv.index("-i")
            argv[i:i] = ["--dge-levels=vector_dynamic_offsets"]
        return _orig(argv, **kw)

    _bu.run_command = _patched
    _PATCHED = True


# ---------------------------------------------------------------- host config
N, E, G, D = 50000, 800000, 64, 8
NPD = N // D          # nodes per device
BLK = 127             # real node slots per block (slot 127 = trash)
NB = (NPD + BLK - 1) // BLK
R = NB * 128          # padded local rows (one 128-tile per block)
R2 = R // 2           # AllGather half (rows)
EPS = 1e-5
BNC = float(1.0 / np.sqrt(1.0 + EPS))
DIMS = [(32, 4, 64), (256, 4, 128), (512, 4, 64)]
# per-layer gathered payload width P (layer1 gathers raw h=x, others gather xs)
PW = [32, 512, 256]
W2 = [p + 16 for p in PW]   # payload + 8 bf16-slots(=4 f32) al_s + 8 al_d

_CACHE = {}


def _fold(Wm, a):
    """[fin, H*C], [H, C] -> [fin, H] : per-head row-sums of W * a."""
    H, C = a.shape
    return (Wm.reshape(-1, H, C) * a[None]).sum(2)


def _grow(n):
    """node id -> row in the split-half AllGathered table."""
    d_, m = n // NPD, n % NPD
    h = m // R2
    return h * (D * R2) + d_ * R2 + (m - h * R2)


def _prep(inputs):
    """Host-side sharding/layout prep (numpy only). Returns (in_maps, T)."""
    src_g = np.asarray(inputs["edge_index"][0], dtype=np.int64)
    dst_g = np.asarray(inputs["edge_index"][1], dtype=np.int64)
    ef = np.asarray(inputs["edge_feature"], dtype=np.float32)
    batch = np.asarray(inputs["batch"], dtype=np.int64)
    x = np.asarray(inputs["x"], dtype=np.float32)

    per_dev = []
    Tmax = 1
    for d in range(D):
        m = (dst_g // NPD) == d
        s, t, f = src_g[m], dst_g[m], ef[m]
        loc = t - d * NPD
        b = loc // BLK
        rel = loc % BLK
        order = np.argsort(b, kind="stable")
        s, f, b, rel = s[order], f[order], b[order], rel[order]
        cnt = np.bincount(b, minlength=NB)
        Tmax = max(Tmax, int(np.ceil(cnt.max() / 128)))
        per_dev.append((s, f, b, rel, cnt))
    T = Tmax

    # shared (replicated) weight-derived constants
    shared = {}
    for li, (fin, H, C) in enumerate(DIMS, 1):
        HC = H * C
        Wm = np.asarray(inputs[f"W{li}"], np.float32)
        Was = _fold(Wm, np.asarray(inputs[f"as{li}"], np.float32))
        Wad = _fold(Wm, np.asarray(inputs[f"ad{li}"], np.float32))
        g = np.asarray(inputs[f"g{li}"], np.float32) * BNC
        Wm = Wm * g[None, :]
        if li == 1:
            shared["Wasd1"] = np.concatenate([Was, Wad], 1).astype(BF16)   # [32,8]
            w1b = np.zeros((128, 256), np.float32)
            for h in range(H):
                w1b[h * 32:(h + 1) * 32, h * C:(h + 1) * C] = \
                    Wm[:, h * C:(h + 1) * C]
            shared["W1sb"] = w1b.astype(BF16)   # block-diag per head [128,256]
        else:
            shared[f"Wext{li}"] = np.concatenate(
                [Wm, Was, Wad], axis=1).astype(BF16)                       # [fin,HC+8]
        b2 = np.asarray(inputs[f"b{li}"], np.float32) * g + \
            np.asarray(inputs[f"bb{li}"], np.float32)
        shared[f"b2{li}"] = np.broadcast_to(b2.astype(BF16), (128, HC)).copy()
    Ae = np.concatenate(
        [_fold(np.asarray(inputs[f"We{li}"], np.float32),
               np.asarray(inputs[f"ae{li}"], np.float32)) for li in (1, 2, 3)],
        axis=1).astype(np.float32)                                         # [6,12]
    shared["Aecat"] = Ae
    shared["io32"] = np.broadcast_to(
        np.arange(128, dtype=np.float32), (128, 128)).copy()
    shared["io64"] = np.broadcast_to(
        np.arange(64, dtype=np.float32), (128, 64)).copy()
    shared["ident"] = np.eye(128, dtype=np.float32)
    shared["Wf1"] = np.asarray(inputs["Wf1"], np.float32)
    shared["Wf2"] = np.asarray(inputs["Wf2"], np.float32)
    shared["bf1r"] = np.broadcast_to(
        np.asarray(inputs["bf1"], np.float32), (64, 32)).copy()
    shared["gfr"] = np.broadcast_to(
        np.asarray(inputs["gf"], np.float32), (64, 32)).copy()
    shared["bbfr"] = np.broadcast_to(
        np.asarray(inputs["bbf"], np.float32), (64, 32)).copy()
    shared["bf2r"] = np.broadcast_to(
        np.asarray(inputs["bf2"], np.float32), (64, 2)).copy()

    in_maps = []
    for d in range(D):
        s, f, b, rel, cnt = per_dev[d]
        idx_s = np.zeros((NB, T * 128), np.int32)
        idx_d = np.zeros((NB, T * 128), np.int32)
        relm = np.full((NB, T * 128), 127.0, np.float32)
        eaT = np.zeros((6, NB * T * 128), np.float32)
        eaR = np.zeros((NB, T * 128, 8), np.float32)
        off = np.concatenate([[0], np.cumsum(cnt)])
        for blk in range(NB):
            e0, e1 = off[blk], off[blk + 1]
            k = e1 - e0
            idx_s[blk, :k] = _grow(s[e0:e1]).astype(np.int32)
            idx_d[blk, :k] = _grow(d * NPD + blk * BLK + rel[e0:e1]) \
                .astype(np.int32)
            relm[blk, :k] = rel[e0:e1].astype(np.float32)
            eaT[:, blk * T * 128: blk * T * 128 + k] = f[e0:e1].T
            eaR[blk, :k, :6] = f[e0:e1]
            eaR[blk, :k, 6] = 1.0
        # per-tile transposed [NB, 128, T] layouts
        tp = lambda a: a.reshape(NB, T, 128).transpose(0, 2, 1)
        bb = np.full((NB, 128, 1), -1.0, np.float32)
        for blk in range(NB):
            lo = blk * BLK
            n = min(BLK, NPD - lo)
            if n > 0:
                bb[blk, :n, 0] = batch[d * NPD + lo: d * NPD + lo + n]
        eaRt = eaR.reshape(NB, T, 128, 8).transpose(0, 2, 1, 3) \
                  .reshape(NB, 128, T * 8).astype(BF16)
        eaRt_i32 = np.ascontiguousarray(eaRt).view(np.uint16) \
            .reshape(NB, 128, T * 4, 2).view(np.uint32).reshape(NB, 128, T * 4) \
            .view(np.int32)
        # record: [src T | dst T | batch 1 | rel T | eaRt(bf16) 4T] as int32
        rec = np.concatenate(
            [tp(idx_s), tp(idx_d), bb.view(np.int32), tp(relm.view(np.int32)),
             eaRt_i32], axis=2
        ).reshape(NB * 128, 7 * T + 1).copy()

        # dense node-major payload rows for layer 1 (x | zeros for als/ald)
        xr = x[d * NPD:(d + 1) * NPD]
        xrow = np.zeros((R, 48), BF16)
        xrow[:NPD, 0:32] = xr.astype(BF16)
        xT = np.zeros((32, R), BF16)
        xT[:, :NPD] = xr.T.astype(BF16)

        im = {"rec": rec, "eaT": np.ascontiguousarray(eaT),
              "xrow": xrow, "xT": xT}
        im.update(shared)
        in_maps.append(im)
    return in_maps, T


# ---------------------------------------------------------------- device prog
def _build(T, dbg=False):
    import concourse.bass as bass
    import concourse.bacc as bacc
    import concourse.mybir as mybir
    import concourse.tile as tile
    from contextlib import ExitStack

    f32 = mybir.dt.float32
    bf16 = mybir.dt.bfloat16
    i32 = mybir.dt.int32
    AO = mybir.AluOpType
    AF = mybir.ActivationFunctionType
    RG = [list(range(D))]
    RECW = 7 * T + 1
    # recfull row: [idx_s T | idx_d T | batch 1 | sall 64T | ale3 12T+12] (i32)
    C_SALL = 2 * T + 1
    C_ALE = C_SALL + 64 * T
    RFW = C_ALE + 12 * T + 12

    nc = bacc.Bacc(None, target_bir_lowering=False, debug=True)

    # ---- I/O
    inp = {}

    def di(name, shape, dt=f32):
        inp[name] = nc.declare_dram_parameter(name, list(shape), dt,
                                              isOutput=False)
        return inp[name]

    di("rec", (NB * 128, RECW), i32)
    di("eaT", (6, NB * T * 128))
    di("xrow", (R, 48), bf16)
    di("xT", (32, R), bf16)
    di("Wasd1", (32, 8), bf16)
    di("W1sb", (128, 256), bf16)
    di("Wext2", (256, 520), bf16)
    di("Wext3", (512, 264), bf16)
    for li, (fin, H, C) in enumerate(DIMS, 1):
        HC = H * C
        di(f"b2{li}", (128, HC), bf16)
    di("Aecat", (6, 12))
    di("io32", (128, 128)); di("io64", (128, 64)); di("ident", (128, 128))
    di("Wf1", (256, 32)); di("Wf2", (32, 2))
    di("bf1r", (64, 32)); di("gfr", (64, 32)); di("bbfr", (64, 32))
    di("bf2r", (64, 2))
    out_d = nc.declare_dram_parameter("out", [64, 2], f32, isOutput=True)

    # ---- internal DRAM
    xe_d = [[nc.dram_tensor(f"xe{l}_{h}", [R2, W2[l - 1]], bf16)
             for h in (0, 1)] for l in (1, 2, 3)]
    xf_d = [nc.dram_tensor(f"xf{l}", [D * R, W2[l - 1]], bf16,
                           addr_space="Shared") for l in (1, 2, 3)]
    hT_d = [None,
            [nc.dram_tensor(f"hT1_{h}", [R2, 256], bf16) for h in (0, 1)],
            [nc.dram_tensor(f"hT2_{h}", [R2, 512], bf16) for h in (0, 1)]]
    RFC = 13 * 128   # rf chunk rows
    rf_d = [nc.dram_tensor(f"recfull{q}", [min(RFC, NB * 128 - q * RFC), RFW],
                           i32) for q in range(4)]
    pool_i = nc.dram_tensor("pool_i", [64, 257], f32)
    pool_o = nc.dram_tensor("pool_o", [64, 257], f32, addr_space="Shared")

    with ExitStack() as ctx:
        tc = ctx.enter_context(tile.TileContext(nc))
        consts = ctx.enter_context(tc.tile_pool(name="consts", bufs=1))
        lay = ctx.enter_context(tc.tile_pool(name="lay", bufs=1))
        sb = ctx.enter_context(tc.tile_pool(name="sb", bufs=2))
        sb2 = ctx.enter_context(tc.tile_pool(name="sb2", bufs=2))
        sbg = ctx.enter_context(tc.tile_pool(name="sbg", bufs=2))
        psb = ctx.enter_context(tc.tile_pool(name="psb", bufs=2, space="PSUM"))
        pss = ctx.enter_context(tc.tile_pool(name="pss", bufs=2, space="PSUM"))
        pst = ctx.enter_context(tc.tile_pool(name="pst", bufs=2, space="PSUM"))

        io32 = consts.tile([128, 128], f32)
        nc.sync.dma_start(out=io32[:], in_=inp["io32"][:])
        io64 = consts.tile([128, 64], f32)
        nc.sync.dma_start(out=io64[:], in_=inp["io64"][:])
        ident = consts.tile([128, 128], f32)
        nc.sync.dma_start(out=ident[:], in_=inp["ident"][:])
        identb = consts.tile([128, 128], bf16)
        nc.vector.tensor_copy(out=identb[:], in_=ident[:])
        onescolb = consts.tile([128, 1], bf16)
        nc.any.memset(onescolb[:], 1.0)
        Aecat = consts.tile([6, 12], f32)
        nc.sync.dma_start(out=Aecat[:], in_=inp["Aecat"][:])
        Wasd1 = consts.tile([32, 8], bf16)
        nc.sync.dma_start(out=Wasd1[:], in_=inp["Wasd1"][:])
        W1sb = consts.tile([128, 256], bf16)
        nc.sync.dma_start(out=W1sb[:], in_=inp["W1sb"][:])
        b2_t = {}
        for li, (fin, H, C) in enumerate(DIMS, 1):
            HC = H * C
            b2_t[li] = lay.tile([128, HC], bf16, tag=f"b2{li}", name=f"b2{li}t")
            nc.sync.dma_start(out=b2_t[li][:], in_=inp[f"b2{li}"][:])
        wsb_t = {}
        for li in (2, 3):
            fin, H, C = DIMS[li - 1]
            HC = H * C
            nkc = fin // 128
            wsb_t[li] = lay.tile([128, nkc * (HC + 8)], bf16, tag=f"wsb{li}", name=f"wsb{li}t")
            for kc in range(nkc):
                nc.sync.dma_start(
                    out=wsb_t[li][:, kc * (HC + 8):(kc + 1) * (HC + 8)],
                    in_=inp[f"Wext{li}"][kc * 128:(kc + 1) * 128, :])

        pool_sb = consts.tile([64, 257], f32)
        nc.any.memset(pool_sb[:], 0.0)

        def rf_rows(st):
            q, r = st // RFC, st % RFC
            return rf_d[q], r

        # copy host idx/batch cols into recfull chunks
        for q in range(4):
            r0 = q * RFC
            nr = min(RFC, NB * 128 - r0)
            nc.scalar.dma_start(out=rf_d[q][:, 0:2 * T + 1],
                                in_=inp["rec"][r0:r0 + nr, 0:2 * T + 1])

        # ---------------- phase A emitters ----------------
        def phaseA1(rt0, nb):
            xs = sb.tile([128, nb * 48], bf16, tag="xsA1", bufs=3)
            nc.sync.dma_start(
                out=xs[:].rearrange("p (j w) -> p j w", j=nb),
                in_=inp["xrow"][rt0 * 128:(rt0 + nb) * 128, :]
                    .rearrange("(j p) w -> p j w", p=128))
            xtt = sb.tile([32, nb * 128], bf16, tag="xtt", bufs=3)
            nc.scalar.dma_start(out=xtt[:],
                                in_=inp["xT"][:, rt0 * 128:(rt0 + nb) * 128])
            p8 = pst.tile([128, nb * 8], f32, tag="ps3")
            for j in range(nb):
                nc.tensor.matmul(p8[:, j * 8:(j + 1) * 8],
                                 xtt[:, j * 128:(j + 1) * 128], Wasd1[:],
                                 start=True, stop=True)
            nc.vector.tensor_copy(
                out=xs[:].rearrange("p (j w) -> p j w", j=nb)[:, :, 32:48]
                    .bitcast(f32),
                in_=p8[:].rearrange("p (j w) -> p j w", j=nb))
            h = rt0 * 128 // R2
            o = rt0 * 128 - h * R2
            nc.sync.dma_start(
                out=xe_d[0][h][o:o + nb * 128, :]
                    .rearrange("(j p) w -> p j w", p=128),
                in_=xs[:].rearrange("p (j w) -> p j w", j=nb))

        def phaseA(li, rt):
            fin, H, C = DIMS[li - 1]
            HC = H * C
            P = PW[li - 1]
            nkc = fin // 128
            wsb = wsb_t[li]
            pxs = psb.tile([128, 512], f32, tag="big")
            p8 = pst.tile([128, 8], f32, tag="ps3")
            hh_ = rt * 128 // R2
            ro = rt * 128 - hh_ * R2
            for kc in range(nkc):
                ht = sb.tile([128, 128], bf16, tag="ht", bufs=8)
                nc.sync.dma_start(
                    out=ht[:],
                    in_=hT_d[li - 1][hh_][ro:ro + 128,
                                          kc * 128:(kc + 1) * 128],
                    transpose=True)
                W0 = kc * (HC + 8)
                nc.tensor.matmul(pxs[:, 0:HC], ht[:], wsb[:, W0:W0 + HC],
                                 start=(kc == 0), stop=(kc == nkc - 1))
                nc.tensor.matmul(p8[:], ht[:], wsb[:, W0 + HC:W0 + HC + 8],
                                 start=(kc == 0), stop=(kc == nkc - 1))
            xs = sb.tile([128, P + 16], bf16, tag="xsA", bufs=4)
            nc.scalar.activation(out=xs[:, 0:P], in_=pxs[:, 0:P], func=AF.Copy)
            nc.scalar.activation(out=xs[:, P:P + 16].bitcast(f32), in_=p8[:],
                                 func=AF.Copy)
            nc.scalar.dma_start(out=xe_d[li - 1][hh_][ro:ro + 128, :],
                                in_=xs[:])

        def ag(li, half):
            xf = xf_d[li - 1]
            nc.gpsimd.collective_compute(
                "AllGather", AO.bypass, replica_groups=RG,
                ins=[xe_d[li - 1][half][:]],
                outs=[xf[half * D * R2:(half + 1) * D * R2, :]])

        # ---------------- block emitter ----------------
        def block(li, i):
            fin, H, C = DIMS[li - 1]
            HC = H * C
            P = PW[li - 1]
            Wp = W2[li - 1]
            scheme_h = (li == 1)
            st128 = i * 128
            stblk = i * BLK
            xe = xe_d[li - 1]
            xf = xf_d[li - 1]
            b2 = b2_t[li]

            rft, ro_ = rf_rows(st128)
            if scheme_h:
                rfl = sb.tile([128, RECW], i32, tag="rfl", bufs=3)
                nc.sync.dma_start(out=rfl[:],
                                  in_=inp["rec"][st128:st128 + 128, :])
                eatt = sb2.tile([6, T * 128], f32, tag="eatt", bufs=3)
                nc.scalar.dma_start(
                    out=eatt[:],
                    in_=inp["eaT"][:, st128 * T:(st128 + 128) * T])
            else:
                rfl = sb.tile([128, RFW], i32, tag="rfl", bufs=3)
                nc.sync.dma_start(out=rfl[:], in_=rft[ro_:ro_ + 128, :])
            xsl = sb.tile([128, Wp], bf16, tag="xsl")
            if stblk + 128 <= R2:
                nc.scalar.dma_start(out=xsl[:], in_=xe[0][stblk:stblk + 128, :])
            elif stblk >= R2:
                nc.scalar.dma_start(out=xsl[:],
                                    in_=xe[1][stblk - R2:stblk - R2 + 128, :])
            else:
                nlo = R2 - stblk
                nc.scalar.dma_start(out=xsl[0:nlo, :], in_=xe[0][stblk:R2, :])
                nc.scalar.dma_start(out=xsl[nlo:128, :],
                                    in_=xe[1][0:128 - nlo, :])
            gat = sbg.tile([128, T * Wp], bf16, tag="gat", bufs=3)
            aldg = sb.tile([128, T * 8], bf16, tag="aldg", bufs=3)
            for t in range(T):
                nc.gpsimd.indirect_dma_start(
                    out=gat[:, t * Wp:(t + 1) * Wp], out_offset=None,
                    in_=xf[:],
                    in_offset=bass.IndirectOffsetOnAxis(ap=rfl[:, t:t + 1],
                                                        axis=0))
                nc.gpsimd.indirect_dma_start(
                    out=aldg[:, t * 8:(t + 1) * 8], out_offset=None,
                    in_=xf[:],
                    in_offset=bass.IndirectOffsetOnAxis(
                        ap=rfl[:, T + t:T + t + 1], axis=0),
                    element_offset=P + 8)
            if scheme_h:
                rel = rfl[:, 2 * T + 1:3 * T + 1].bitcast(f32)
                sall_t = sbg.tile([128, T * 128], bf16, tag="salle", bufs=3)
                nc.vector.tensor_tensor(
                    out=sall_t[:].rearrange("p (t n) -> p t n", t=T),
                    in0=rel.unsqueeze(2).to_broadcast([128, T, 128]),
                    in1=io32[:].unsqueeze(1).to_broadcast([128, T, 128]),
                    op=AO.is_equal)
                sall = sall_t[:]
                nc.sync.dma_start(out=rft[ro_:ro_ + 128, C_SALL:C_ALE],
                                  in_=sall_t[:].bitcast(i32))
                # al_e for all 3 layers (edges) via PE from transposed ea
                alp = pss.tile([128, 12 * T], f32, tag="sm1")
                for t in range(T):
                    nc.tensor.matmul(alp[:, t * 12:(t + 1) * 12],
                                     eatt[:, t * 128:(t + 1) * 128],
                                     Aecat[:], start=True, stop=True)
                ale = sb.tile([128, 12 * T + 12], f32, tag="ale", bufs=3)
                a3 = ale[:].rearrange("p (l x) -> p l x", l=3)
                nc.vector.tensor_copy(
                    out=a3[:, :, 0:4 * T].rearrange("p l (t h) -> p l t h",
                                                    t=T),
                    in_=alp[:].rearrange("p (t l h) -> p l t h", l=3, h=4))
                alet = None
            else:
                sall = rfl[:, C_SALL:C_ALE].bitcast(bf16)
                alet = rfl[:, C_ALE + (li - 1) * (4 * T + 4):
                           C_ALE + li * (4 * T + 4)].bitcast(f32)
            # logits (fp32): al_s[src] + al_d[dst] + al_e
            alsw = sb.tile([128, T * 16], bf16, tag="alsw")
            nc.vector.tensor_copy(
                out=alsw[:].rearrange("p (t w) -> p t w", t=T),
                in_=gat[:].rearrange("p (t w) -> p t w", t=T)[:, :, P:P + 16])
            wall = sb.tile([128, T * 4], f32, tag="wall")
            nc.vector.tensor_tensor(
                out=wall[:].rearrange("p (t h) -> p t h", t=T),
                in0=alsw[:].bitcast(f32).rearrange(
                    "p (t w) -> p t w", t=T)[:, :, 0:4],
                in1=aldg[:].bitcast(f32).rearrange(
                    "p (t w) -> p t w", t=T)[:, :, 0:4],
                op=AO.add)
            ale_e = a3[:, 0, 0:4 * T] if scheme_h else alet[:, 0:4 * T]
            nc.vector.tensor_tensor(out=wall[:], in0=wall[:],
                                    in1=ale_e, op=AO.add)
            lk = sb.tile([128, T * 4], f32, tag="lk")
            nc.vector.tensor_scalar(out=lk[:], in0=wall[:], scalar1=0.2,
                                    scalar2=None, op0=AO.mult)
            nc.vector.tensor_tensor(out=wall[:], in0=wall[:], in1=lk[:],
                                    op=AO.max)
            wallb = sb.tile([128, T * 4], bf16, tag="wallb")
            nc.scalar.activation(out=wallb[:], in_=wall[:], func=AF.Exp)
            # values: w * payload (+ w in trailing cols for denominator)
            if scheme_h:
                VW = 140
                val = sbg.tile([128, T * VW], bf16, tag="val")
                v3 = val[:].rearrange("p (t w) -> p t w", t=T)
                nc.vector.tensor_tensor(
                    out=v3[:, :, 0:128].rearrange("p t (h c) -> p t h c", h=4),
                    in0=gat[:].rearrange("p (t w) -> p t w", t=T)
                        [:, :, 0:32].unsqueeze(2).to_broadcast([128, T, 4, 32]),
                    in1=wallb[:].rearrange("p (t h) -> p t h", t=T)
                        .unsqueeze(3).to_broadcast([128, T, 4, 32]),
                    op=AO.mult)
                nc.vector.tensor_copy(
                    out=v3[:, :, 128:132],
                    in_=wallb[:].rearrange("p (t h) -> p t h", t=T))
                nc.vector.tensor_copy(
                    out=v3[:, :, 132:140],
                    in_=rfl[:, 3 * T + 1:7 * T + 1].bitcast(bf16)
                        .rearrange("p (t w) -> p t w", t=T))
                nps = pss.tile([128, VW], f32, tag="sm")
                for t in range(T):
                    nc.tensor.matmul(nps[:], sall[:, t * 128:(t + 1) * 128],
                                     val[:, t * VW:(t + 1) * VW],
                                     start=(t == 0), stop=(t == T - 1))
                # emean + self-loop / edge al_e epilogue rows (cols 132:140
                # hold one-hot-scattered [ea|deg] sums)
                degr = sb.tile([128, 1], f32, tag="degr")
                nc.vector.tensor_scalar(out=degr[:], in0=nps[:, 138:139],
                                        scalar1=1.0, scalar2=None, op0=AO.max)
                nc.vector.reciprocal(out=degr[:], in_=degr[:])
                em = sb.tile([128, 8], f32, tag="em")
                nc.vector.tensor_scalar(out=em[:, 0:6], in0=nps[:, 132:138],
                                        scalar1=degr[:], scalar2=None,
                                        op0=AO.mult)
                emt_ps = pst.tile([6, 128], f32, tag="ps3")
                nc.tensor.transpose(emt_ps[:], em[:, 0:6], ident[:])
                emt = sb.tile([6, 128], f32, tag="emts")
                nc.vector.tensor_copy(out=emt[:], in_=emt_ps[:])
                asps = pst.tile([128, 12], f32, tag="ps3")
                nc.tensor.matmul(asps[:], emt[:], Aecat[:],
                                 start=True, stop=True)
                nc.vector.tensor_copy(
                    out=a3[:, :, 4 * T:4 * T + 4],
                    in_=asps[:].rearrange("p (l h) -> p l h", l=3))
                nc.scalar.dma_start(out=rft[ro_:ro_ + 128, C_ALE:RFW],
                                    in_=ale[:].bitcast(i32))
            else:
                g3 = gat[:].rearrange("p (t w) -> p t w", t=T)
                nc.vector.tensor_tensor(
                    out=g3[:, :, 0:P].rearrange("p t (h c) -> p t h c", h=4),
                    in0=g3[:, :, 0:P].rearrange("p t (h c) -> p t h c", h=4),
                    in1=wallb[:].rearrange("p (t h) -> p t h", t=T)
                        .unsqueeze(3).to_broadcast([128, T, 4, C]),
                    op=AO.mult)
                nc.vector.tensor_copy(
                    out=g3[:, :, P:P + 4],
                    in_=wallb[:].rearrange("p (t h) -> p t h", t=T))
                if li == 2:
                    nps1 = pss.tile([128, 256], f32, tag="sm1")
                    nps = pss.tile([128, 260], f32, tag="sm")
                    for t in range(T):
                        nc.tensor.matmul(nps1[:],
                                         sall[:, t * 128:(t + 1) * 128],
                                         gat[:, t * Wp:t * Wp + 256],
                                         start=(t == 0), stop=(t == T - 1))
                        nc.tensor.matmul(nps[:],
                                         sall[:, t * 128:(t + 1) * 128],
                                         gat[:, t * Wp + 256:t * Wp + 516],
                                         start=(t == 0), stop=(t == T - 1))
                else:
                    nps = pss.tile([128, 260], f32, tag="sm")
                    for t in range(T):
                        nc.tensor.matmul(nps[:],
                                         sall[:, t * 128:(t + 1) * 128],
                                         gat[:, t * Wp:t * Wp + 260],
                                         start=(t == 0), stop=(t == T - 1))
            # self-loop weight
            u = xsl[:, P:P + 16].bitcast(f32)
            sl0 = sb.tile([128, 4], f32, tag="sl0")
            nc.vector.tensor_tensor(out=sl0[:], in0=u[:, 0:4], in1=u[:, 4:8],
                                    op=AO.add)
            ale_s = a3[:, 0, 4 * T:4 * T + 4] if scheme_h \
                else alet[:, 4 * T:4 * T + 4]
            nc.vector.tensor_tensor(out=sl0[:], in0=sl0[:],
                                    in1=ale_s, op=AO.add)
            lk2 = sb.tile([128, 4], f32, tag="lk2")
            nc.vector.tensor_scalar(out=lk2[:], in0=sl0[:], scalar1=0.2,
                                    scalar2=None, op0=AO.mult)
            nc.vector.tensor_tensor(out=sl0[:], in0=sl0[:], in1=lk2[:],
                                    op=AO.max)
            sl = sb.tile([128, 4], f32, tag="sl")
            nc.scalar.activation(out=sl[:], in_=sl0[:], func=AF.Exp)
            slb = sb.tile([128, 4], bf16, tag="slb")
            nc.vector.tensor_copy(out=slb[:], in_=sl[:])
            # denominator
            NPSc = 132 if scheme_h else 260
            den = sb.tile([128, 4], f32, tag="den")
            nc.vector.tensor_tensor(out=den[:], in0=nps[:, NPSc - 4:NPSc],
                                    in1=sl[:], op=AO.add)
            nc.vector.reciprocal(out=den[:], in_=den[:])
            denb = sb.tile([128, 4], bf16, tag="denb")
            nc.vector.tensor_copy(out=denb[:], in_=den[:])

            hh = sb2.tile([128, HC], bf16, tag="hh")
            if scheme_h:
                aggb = sb.tile([128, 128], bf16, tag="aggb")
                nc.scalar.activation(out=aggb[:], in_=nps[:, 0:128],
                                     func=AF.Copy)
                tmp = sb.tile([128, 128], bf16, tag="tmpL1")
                nc.vector.tensor_tensor(
                    out=tmp[:].rearrange("p (h c) -> p h c", h=4),
                    in0=xsl[:, 0:32].unsqueeze(1).to_broadcast([128, 4, 32]),
                    in1=slb[:].unsqueeze(2).to_broadcast([128, 4, 32]),
                    op=AO.mult)
                nc.vector.tensor_tensor(out=aggb[:], in0=aggb[:], in1=tmp[:],
                                        op=AO.add)
                nc.vector.tensor_tensor(
                    out=aggb[:].rearrange("p (h c) -> p h c", h=4),
                    in0=aggb[:].rearrange("p (h c) -> p h c", h=4),
                    in1=denb[:].unsqueeze(2).to_broadcast([128, 4, 32]),
                    op=AO.mult)
                agT_ps = pst.tile([128, 128], bf16, tag="ps3")
                nc.tensor.transpose(agT_ps[:], aggb[:], identb[:])
                agT = sb.tile([128, 128], bf16, tag="agT")
                nc.vector.tensor_copy(out=agT[:], in_=agT_ps[:])
                hps = psb.tile([128, 512], f32, tag="big")
                nc.tensor.matmul(hps[:, 0:256], agT[:], W1sb[:],
                                 start=True, stop=True)
                hpre = sb.tile([128, HC], bf16, tag="hpre")
                nc.scalar.activation(out=hpre[:], in_=hps[:, 0:256],
                                     func=AF.Copy)
            else:
                aggb = sb.tile([128, HC], bf16, tag="aggb")
                if li == 2:
                    nc.scalar.activation(out=aggb[:, 0:256], in_=nps1[:],
                                         func=AF.Copy)
                    nc.scalar.activation(out=aggb[:, 256:512],
                                         in_=nps[:, 0:256], func=AF.Copy)
                else:
                    nc.scalar.activation(out=aggb[:], in_=nps[:, 0:HC],
                                         func=AF.Copy)
                tmp = sb2.tile([128, HC], bf16, tag="tmpL")
                nc.vector.tensor_tensor(
                    out=tmp[:].rearrange("p (h c) -> p h c", h=4),
                    in0=xsl[:, 0:P].rearrange("p (h c) -> p h c", h=4),
                    in1=slb[:].unsqueeze(2).to_broadcast([128, 4, C]),
                    op=AO.mult)
                nc.vector.tensor_tensor(out=aggb[:], in0=aggb[:], in1=tmp[:],
                                        op=AO.add)
                hpre = aggb
                nc.vector.tensor_tensor(
                    out=hpre[:].rearrange("p (h c) -> p h c", h=4),
                    in0=hpre[:].rearrange("p (h c) -> p h c", h=4),
                    in1=denb[:].unsqueeze(2).to_broadcast([128, 4, C]),
                    op=AO.mult)
            # BN bias (scale folded into W on host) + ELU
            nc.vector.tensor_tensor(out=hpre[:], in0=hpre[:], in1=b2[:],
                                    op=AO.add)
            zn = sb2.tile([128, HC], bf16, tag="zn")
            nc.vector.tensor_scalar(out=zn[:], in0=hpre[:], scalar1=0.0,
                                    scalar2=None, op0=AO.min)
            nc.scalar.activation(out=zn[:], in_=zn[:], func=AF.Exp)
            rl = sb2.tile([128, HC], bf16, tag="rl")
            nc.vector.tensor_scalar(out=rl[:], in0=hpre[:], scalar1=0.0,
                                    scalar2=None, op0=AO.max)
            nc.vector.scalar_tensor_tensor(
                out=hh[:], in0=zn[:], scalar=-1.0, in1=rl[:],
                op0=AO.add, op1=AO.add)
            if li < 3:
                if stblk + BLK <= R2:
                    nc.sync.dma_start(out=hT_d[li][0][stblk:stblk + BLK, :],
                                      in_=hh[0:BLK, :])
                elif stblk >= R2:
                    nc.sync.dma_start(
                        out=hT_d[li][1][stblk - R2:stblk - R2 + BLK, :],
                        in_=hh[0:BLK, :])
                else:
                    nlo = R2 - stblk
                    nc.sync.dma_start(out=hT_d[li][0][stblk:R2, :],
                                      in_=hh[0:nlo, :])
                    nc.sync.dma_start(out=hT_d[li][1][0:BLK - nlo, :],
                                      in_=hh[nlo:BLK, :])
            else:
                bcol = rfl[:, 2 * T:2 * T + 1].bitcast(f32)
                bt = sb.tile([128, 64], bf16, tag="bt")
                nc.vector.tensor_tensor(out=bt[:],
                                        in0=bcol.to_broadcast([128, 64]),
                                        in1=io64[:], op=AO.is_equal)
                pps = pst.tile([64, 257], f32, tag="ps3")
                nc.tensor.matmul(pps[:, 0:HC], bt[:], hh[:],
                                 start=True, stop=True)
                nc.tensor.matmul(pps[:, HC:HC + 1], bt[:], onescolb[:],
                                 start=True, stop=True)
                nc.vector.tensor_tensor(out=pool_sb[:], in0=pool_sb[:],
                                        in1=pps[:], op=AO.add)

        # ---------------- program ----------------
        for rt0 in range(0, 25, 5):
            phaseA1(rt0, 5)
        ag(1, 0)
        for rt0 in range(25, NB, 5):
            phaseA1(rt0, 5)
        ag(1, 1)

        # ---- block loops; next-layer linear tiles emitted in two batches ----
        # (in-order engine queues: emit work only once its deps are ~ready)
        for li in (1, 2, 3):
            for i in range(NB):
                block(li, i)
                if li < 3:
                    if i == 25:          # hT lo-half complete after block 25
                        for rt in range(0, 25):
                            phaseA(li + 1, rt)
                    if i == 41:
                        ag(li + 1, 0)
                    if i == NB - 1:      # hi-half hT complete
                        for rt in range(25, NB):
                            phaseA(li + 1, rt)
                        ag(li + 1, 1)

        # ---------------- final MLP ----------------
        nc.sync.dma_start(out=pool_i[:], in_=pool_sb[:])
        nc.gpsimd.collective_compute("AllReduce", AO.add, replica_groups=RG,
                                     ins=[pool_i[:]], outs=[pool_o[:]])
        pool2 = sb.tile([64, 257], f32, tag="pool2")
        nc.sync.dma_start(out=pool2[:], in_=pool_o[:])
        cnt = sb.tile([64, 1], f32, tag="cnt")
        nc.vector.tensor_scalar(out=cnt[:], in0=pool2[:, 256:257], scalar1=1.0,
                                scalar2=None, op0=AO.max)
        nc.vector.reciprocal(out=cnt[:], in_=cnt[:])
        nc.vector.tensor_scalar(out=pool2[:, 0:256], in0=pool2[:, 0:256],
                                scalar1=cnt[:], scalar2=None, op0=AO.mult)
        pts = sb.tile([128, 128], f32, tag="pts")
        for ch in range(2):
            ptp = pst.tile([128, 64], f32, tag="ps3")
            nc.tensor.transpose(ptp[:], pool2[:, ch * 128:(ch + 1) * 128],
                                ident[0:64, 0:64])
            nc.vector.tensor_copy(out=pts[:, ch * 64:(ch + 1) * 64],
                                  in_=ptp[:])
        wf1 = sb.tile([128, 64], f32, tag="wf1")
        for ch in range(2):
            nc.sync.dma_start(out=wf1[:, ch * 32:(ch + 1) * 32],
                              in_=inp["Wf1"][ch * 128:(ch + 1) * 128, :])
        z1p = pst.tile([64, 32], f32, tag="ps3")
        for ch in range(2):
            nc.tensor.matmul(z1p[:], pts[:, ch * 64:(ch + 1) * 64],
                             wf1[:, ch * 32:(ch + 1) * 32],
                             start=(ch == 0), stop=(ch == 1))
        gf = sb.tile([64, 32], f32, tag="gf")
        nc.sync.dma_start(out=gf[:], in_=inp["gfr"][:])
        nc.vector.tensor_scalar(out=gf[:], in0=gf[:], scalar1=BNC,
                                scalar2=None, op0=AO.mult)
        b2f = sb.tile([64, 32], f32, tag="b2f")
        nc.sync.dma_start(out=b2f[:], in_=inp["bf1r"][:])
        nc.vector.tensor_tensor(out=b2f[:], in0=b2f[:], in1=gf[:], op=AO.mult)
        bbf = sb.tile([64, 32], f32, tag="bbf")
        nc.sync.dma_start(out=bbf[:], in_=inp["bbfr"][:])
        nc.vector.tensor_tensor(out=b2f[:], in0=b2f[:], in1=bbf[:], op=AO.add)
        zf = sb.tile([64, 32], f32, tag="zf")
        nc.vector.tensor_tensor(out=zf[:], in0=z1p[:], in1=gf[:], op=AO.mult)
        nc.vector.tensor_tensor(out=zf[:], in0=zf[:], in1=b2f[:], op=AO.add)
        zn2 = sb.tile([64, 32], f32, tag="zn2")
        nc.vector.tensor_scalar(out=zn2[:], in0=zf[:], scalar1=0.0,
                                scalar2=None, op0=AO.min)
        nc.scalar.activation(out=zn2[:], in_=zn2[:], func=AF.Exp)
        rl2 = sb.tile([64, 32], f32, tag="rl2")
        nc.vector.tensor_scalar(out=rl2[:], in0=zf[:], scalar1=0.0,
                                scalar2=None, op0=AO.max)
        nc.vector.scalar_tensor_tensor(out=zf[:], in0=zn2[:], scalar=-1.0,
                                       in1=rl2[:], op0=AO.add, op1=AO.add)
        ztp = pst.tile([32, 64], f32, tag="ps3")
        nc.tensor.transpose(ztp[:], zf[:], ident[0:64, 0:64])
        zts = sb.tile([32, 64], f32, tag="zts")
        nc.vector.tensor_copy(out=zts[:], in_=ztp[:])
        wf2 = sb.tile([32, 2], f32, tag="wf2")
        nc.sync.dma_start(out=wf2[:], in_=inp["Wf2"][:])
        z2p = pst.tile([64, 2], f32, tag="ps3")
        nc.tensor.matmul(z2p[:], zts[:], wf2[:], start=True, stop=True)
        bf2 = sb.tile([64, 2], f32, tag="bf2")
        nc.sync.dma_start(out=bf2[:], in_=inp["bf2r"][:])
        z2 = sb.tile([64, 2], f32, tag="z2")
        nc.vector.tensor_tensor(out=z2[:], in0=z2p[:], in1=bf2[:], op=AO.add)
        mrow = sb.tile([64, 1], f32, tag="mrow")
        nc.vector.tensor_reduce(out=mrow[:], in_=z2[:],
                                axis=mybir.AxisListType.X, op=AO.max)
        nc.vector.tensor_scalar(out=z2[:], in0=z2[:], scalar1=mrow[:],
                                scalar2=None, op0=AO.subtract)
        ez = sb.tile([64, 2], f32, tag="ez")
        nc.scalar.activation(out=ez[:], in_=z2[:], func=AF.Exp)
        ssum = sb.tile([64, 1], f32, tag="ssum")
        nc.vector.tensor_reduce(out=ssum[:], in_=ez[:],
                                axis=mybir.AxisListType.X, op=AO.add)
        nc.scalar.activation(out=ssum[:], in_=ssum[:], func=AF.Ln)
        nc.vector.tensor_scalar(out=z2[:], in0=z2[:], scalar1=ssum[:],
                                scalar2=None, op0=AO.subtract)
        nc.sync.dma_start(out=out_d[:, :], in_=z2[:])

    nc.compile()
    return nc


# ---------------------------------------------------------------- entry point
def kernel(**inputs):
    _patch_walrus()
    in_maps, T = _prep(inputs)
    if T not in _CACHE:
        _CACHE[T] = _build(T)
    nc = _CACHE[T]
    from concourse.bass_utils import run_bass_kernel_spmd
    res = run_bass_kernel_spmd(nc, in_maps, list(range(D))).results
    return np.asarray(res[0]["out"], dtype=np.float32)


# revision 3
# speedup vs baseline: 1.0186x; 1.0186x over previous
"""Trainium2 Bass kernel for nn_GAT_n2v_mean (3-layer edge-featured GAT + mean-pool + MLP).

v2 strategy: edges partitioned by dst owner across 8 cores (6250 nodes each,
127-node blocks + trash slot). Per block, ONE batched indirect-DMA gathers all
T*128 src payload rows out of an AllGathered bf16 node table; a second tiny
batched gather fetches per-edge dst attention terms. Segment softmax needs no
max shift (a global shift cancels in the normalization; logits are O(1) for
this input distribution). Attention logits stay fp32 (al_s/al_d ride in the
bf16 table as bit-cast fp32 pairs); values and aggregation matmuls are bf16.
Layer 1 aggregates raw 32-dim inputs and applies W after aggregation.
Per-edge one-hot scatter masks and al_e terms are precomputed on device once
(overlapped with the first AllGather) and DMA'd back per layer. Each
AllGather is split in halves, with next-layer linear tiles interleaved into
the current block loop so the first half overlaps block compute.
"""

import numpy as np
import ml_dtypes

BF16 = ml_dtypes.bfloat16

_PATCHED = False


def _patch_walrus():
    """Enable per-partition vector dynamic offsets in walrus codegen
    (needed for the indirect row gathers; off by default in this path)."""
    global _PATCHED
    if _PATCHED:
        return
    import concourse.bass_utils as _bu
    _orig = _bu.run_command

    def _patched(argv, **kw):
        if any("codegen" in str(a) for a in argv):
            argv = list(argv)
            i = argv.index("-i")
            argv[i:i] = ["--dge-levels=vector_dynamic_offsets"]
        return _orig(argv, **kw)

    _bu.run_command = _patched
    _PATCHED = True


# ---------------------------------------------------------------- host config
N, E, G, D = 50000, 800000, 64, 8
NPD = N // D          # nodes per device
BLK = 127             # real node slots per block (slot 127 = trash)
NB = (NPD + BLK - 1) // BLK
R = NB * 128          # padded local rows (one 128-tile per block)
R2 = R // 2           # AllGather half (rows)
EPS = 1e-5
BNC = float(1.0 / np.sqrt(1.0 + EPS))
DIMS = [(32, 4, 64), (256, 4, 128), (512, 4, 64)]
# per-layer gathered payload width P (layer1 gathers raw h=x, others gather xs)
PW = [32, 512, 256]
# payload + 8 bf16-slots(=4 f32) al_s + 8 al_d + 4 true-bf16 al_d
W2 = [p + 20 for p in PW]

_CACHE = {}


def _fold(Wm, a):
    """[fin, H*C], [H, C] -> [fin, H] : per-head row-sums of W * a."""
    H, C = a.shape
    return (Wm.reshape(-1, H, C) * a[None]).sum(2)


def _grow(n):
    """node id -> row in the split-half AllGathered table."""
    d_, m = n // NPD, n % NPD
    h = m // R2
    return h * (D * R2) + d_ * R2 + (m - h * R2)


def _prep(inputs):
    """Host-side sharding/layout prep (numpy only). Returns (in_maps, T)."""
    src_g = np.asarray(inputs["edge_index"][0], dtype=np.int64)
    dst_g = np.asarray(inputs["edge_index"][1], dtype=np.int64)
    ef = np.asarray(inputs["edge_feature"], dtype=np.float32)
    batch = np.asarray(inputs["batch"], dtype=np.int64)
    x = np.asarray(inputs["x"], dtype=np.float32)

    per_dev = []
    Tmax = 1
    for d in range(D):
        m = (dst_g // NPD) == d
        s, t, f = src_g[m], dst_g[m], ef[m]
        loc = t - d * NPD
        b = loc // BLK
        rel = loc % BLK
        order = np.argsort(b, kind="stable")
        s, f, b, rel = s[order], f[order], b[order], rel[order]
        cnt = np.bincount(b, minlength=NB)
        Tmax = max(Tmax, int(np.ceil(cnt.max() / 128)))
        per_dev.append((s, f, b, rel, cnt))
    T = Tmax

    # shared (replicated) weight-derived constants
    shared = {}
    for li, (fin, H, C) in enumerate(DIMS, 1):
        HC = H * C
        Wm = np.asarray(inputs[f"W{li}"], np.float32)
        Was = _fold(Wm, np.asarray(inputs[f"as{li}"], np.float32))
        Wad = _fold(Wm, np.asarray(inputs[f"ad{li}"], np.float32))
        g = np.asarray(inputs[f"g{li}"], np.float32) * BNC
        Wm = Wm * g[None, :]
        if li == 1:
            shared["Wasd1"] = np.concatenate([Was, Wad], 1).astype(BF16)   # [32,8]
            w1b = np.zeros((128, 256), np.float32)
            for h in range(H):
                w1b[h * 32:(h + 1) * 32, h * C:(h + 1) * C] = \
                    Wm[:, h * C:(h + 1) * C]
            shared["W1sb"] = w1b.astype(BF16)   # block-diag per head [128,256]
        else:
            shared[f"Wext{li}"] = np.concatenate(
                [Wm, Was, Wad], axis=1).astype(BF16)                       # [fin,HC+8]
        b2 = np.asarray(inputs[f"b{li}"], np.float32) * g + \
            np.asarray(inputs[f"bb{li}"], np.float32)
        shared[f"b2{li}"] = np.broadcast_to(b2.astype(BF16), (128, HC)).copy()
    Ae = np.concatenate(
        [_fold(np.asarray(inputs[f"We{li}"], np.float32),
               np.asarray(inputs[f"ae{li}"], np.float32)) for li in (1, 2, 3)],
        axis=1).astype(np.float32)                                         # [6,12]
    shared["Aecat"] = Ae
    shared["io32"] = np.broadcast_to(
        np.arange(128, dtype=np.float32), (128, 128)).copy()
    shared["io64"] = np.broadcast_to(
        np.arange(64, dtype=np.float32), (128, 64)).copy()
    shared["ident"] = np.eye(128, dtype=np.float32)
    shared["Wf1"] = np.asarray(inputs["Wf1"], np.float32)
    shared["Wf2"] = np.asarray(inputs["Wf2"], np.float32)
    shared["bf1r"] = np.broadcast_to(
        np.asarray(inputs["bf1"], np.float32), (64, 32)).copy()
    shared["gfr"] = np.broadcast_to(
        np.asarray(inputs["gf"], np.float32), (64, 32)).copy()
    shared["bbfr"] = np.broadcast_to(
        np.asarray(inputs["bbf"], np.float32), (64, 32)).copy()
    shared["bf2r"] = np.broadcast_to(
        np.asarray(inputs["bf2"], np.float32), (64, 2)).copy()

    in_maps = []
    for d in range(D):
        s, f, b, rel, cnt = per_dev[d]
        idx_s = np.zeros((NB, T * 128), np.int32)
        idx_d = np.zeros((NB, T * 128), np.int32)
        relm = np.full((NB, T * 128), 127.0, np.float32)
        eaT = np.zeros((6, NB * T * 128), np.float32)
        eaR = np.zeros((NB, T * 128, 8), np.float32)
        off = np.concatenate([[0], np.cumsum(cnt)])
        for blk in range(NB):
            e0, e1 = off[blk], off[blk + 1]
            k = e1 - e0
            idx_s[blk, :k] = _grow(s[e0:e1]).astype(np.int32)
            idx_d[blk, :k] = _grow(d * NPD + blk * BLK + rel[e0:e1]) \
                .astype(np.int32)
            relm[blk, :k] = rel[e0:e1].astype(np.float32)
            eaT[:, blk * T * 128: blk * T * 128 + k] = f[e0:e1].T
            eaR[blk, :k, :6] = f[e0:e1]
            eaR[blk, :k, 6] = 1.0
        # per-tile transposed [NB, 128, T] layouts
        tp = lambda a: a.reshape(NB, T, 128).transpose(0, 2, 1)
        bb = np.full((NB, 128, 1), -1.0, np.float32)
        for blk in range(NB):
            lo = blk * BLK
            n = min(BLK, NPD - lo)
            if n > 0:
                bb[blk, :n, 0] = batch[d * NPD + lo: d * NPD + lo + n]
        eaRt = eaR.reshape(NB, T, 128, 8).transpose(0, 2, 1, 3) \
                  .reshape(NB, 128, T * 8).astype(BF16)
        eaRt_i32 = np.ascontiguousarray(eaRt).view(np.uint16) \
            .reshape(NB, 128, T * 4, 2).view(np.uint32).reshape(NB, 128, T * 4) \
            .view(np.int32)
        # transposed one-hot scatter masks, host-built: sllT[n, t*128+e]
        sllT = np.zeros((NB, 128, T * 128), BF16)
        for blk in range(NB):
            rr = relm[blk].astype(np.int64)          # [T*128] slot per edge
            kk = np.arange(T * 128)
            # edge flat j lives at tile t=j//128, lane p=j%128 -> col t*128+p
            sllT[blk, rr, (kk // 128) * 128 + (kk % 128)] = 1.0
        sllT_i32 = np.ascontiguousarray(sllT).view(np.uint16) \
            .reshape(NB, 128, T * 64, 2).view(np.uint32) \
            .reshape(NB, 128, T * 64).view(np.int32)
        # record: [src T | dst T | batch 1 | rel T | eaRt 4T | sllT 64T] i32
        rec = np.concatenate(
            [tp(idx_s), tp(idx_d), bb.view(np.int32), tp(relm.view(np.int32)),
             eaRt_i32, sllT_i32], axis=2
        ).reshape(NB * 128, 71 * T + 1).copy()

        # dense node-major payload rows for layer 1 (x | zeros for als/ald)
        xr = x[d * NPD:(d + 1) * NPD]
        xrow = np.zeros((R, 52), BF16)
        xrow[:NPD, 0:32] = xr.astype(BF16)
        xT = np.zeros((32, R), BF16)
        xT[:, :NPD] = xr.T.astype(BF16)

        im = {"rec": rec, "eaT": np.ascontiguousarray(eaT),
              "xrow": xrow, "xT": xT}
        im.update(shared)
        in_maps.append(im)
    return in_maps, T


# ---------------------------------------------------------------- device prog
def _build(T, dbg=False):
    import concourse.bass as bass
    import concourse.bacc as bacc
    import concourse.mybir as mybir
    import concourse.tile as tile
    from contextlib import ExitStack

    f32 = mybir.dt.float32
    bf16 = mybir.dt.bfloat16
    i32 = mybir.dt.int32
    AO = mybir.AluOpType
    AF = mybir.ActivationFunctionType
    RG = [list(range(D))]
    RECW = 71 * T + 1
    # recfull row: [idx_s T | idx_d T | batch 1 | sall 64T | ale3 12T+12] (i32)
    C_SALL = 2 * T + 1
    C_ALE = C_SALL + 64 * T
    RFW = C_ALE + 12 * T + 12

    nc = bacc.Bacc(None, target_bir_lowering=False, debug=True)

    # ---- I/O
    inp = {}

    def di(name, shape, dt=f32):
        inp[name] = nc.declare_dram_parameter(name, list(shape), dt,
                                              isOutput=False)
        return inp[name]

    di("rec", (NB * 128, RECW), i32)
    C_SLLT = 7 * T + 1
    di("eaT", (6, NB * T * 128))
    di("xrow", (R, 52), bf16)
    di("xT", (32, R), bf16)
    di("Wasd1", (32, 8), bf16)
    di("W1sb", (128, 256), bf16)
    di("Wext2", (256, 520), bf16)
    di("Wext3", (512, 264), bf16)
    for li, (fin, H, C) in enumerate(DIMS, 1):
        HC = H * C
        di(f"b2{li}", (128, HC), bf16)
    di("Aecat", (6, 12))
    di("io32", (128, 128)); di("io64", (128, 64)); di("ident", (128, 128))
    di("Wf1", (256, 32)); di("Wf2", (32, 2))
    di("bf1r", (64, 32)); di("gfr", (64, 32)); di("bbfr", (64, 32))
    di("bf2r", (64, 2))
    out_d = nc.declare_dram_parameter("out", [64, 2], f32, isOutput=True)

    # ---- internal DRAM
    xe_d = [[nc.dram_tensor(f"xe{l}_{h}", [R2, W2[l - 1]], bf16)
             for h in (0, 1)] for l in (1, 2, 3)]
    xf_d = [nc.dram_tensor(f"xf{l}", [D * R, W2[l - 1]], bf16,
                           addr_space="Shared") for l in (1, 2, 3)]
    hT_d = [None,
            [nc.dram_tensor(f"hT1_{h}", [R2, 256], bf16) for h in (0, 1)],
            [nc.dram_tensor(f"hT2_{h}", [R2, 512], bf16) for h in (0, 1)]]
    RFC = 13 * 128   # rf chunk rows
    rf_d = [nc.dram_tensor(f"recfull{q}", [min(RFC, NB * 128 - q * RFC), RFW],
                           i32) for q in range(4)]
    pool_i = nc.dram_tensor("pool_i", [64, 257], f32)
    pool_o = nc.dram_tensor("pool_o", [64, 257], f32, addr_space="Shared")

    with ExitStack() as ctx:
        tc = ctx.enter_context(tile.TileContext(nc))
        consts = ctx.enter_context(tc.tile_pool(name="consts", bufs=1))
        lay = ctx.enter_context(tc.tile_pool(name="lay", bufs=1))
        sb = ctx.enter_context(tc.tile_pool(name="sb", bufs=2))
        sb2 = ctx.enter_context(tc.tile_pool(name="sb2", bufs=2))
        sbg = ctx.enter_context(tc.tile_pool(name="sbg", bufs=2))
        psb = ctx.enter_context(tc.tile_pool(name="psb", bufs=2, space="PSUM"))
        pss = ctx.enter_context(tc.tile_pool(name="pss", bufs=2, space="PSUM"))
        pst = ctx.enter_context(tc.tile_pool(name="pst", bufs=2, space="PSUM"))

        io32 = consts.tile([128, 128], f32)
        nc.sync.dma_start(out=io32[:], in_=inp["io32"][:])
        io64 = consts.tile([128, 64], f32)
        nc.sync.dma_start(out=io64[:], in_=inp["io64"][:])
        ident = consts.tile([128, 128], f32)
        nc.sync.dma_start(out=ident[:], in_=inp["ident"][:])
        identb = consts.tile([128, 128], bf16)
        nc.vector.tensor_copy(out=identb[:], in_=ident[:])
        onescolb = consts.tile([128, 1], bf16)
        nc.any.memset(onescolb[:], 1.0)
        Aecat = consts.tile([6, 12], f32)
        nc.sync.dma_start(out=Aecat[:], in_=inp["Aecat"][:])
        Wasd1 = consts.tile([32, 8], bf16)
        nc.sync.dma_start(out=Wasd1[:], in_=inp["Wasd1"][:])
        W1sb = consts.tile([128, 256], bf16)
        nc.sync.dma_start(out=W1sb[:], in_=inp["W1sb"][:])
        b2_t = {}
        for li, (fin, H, C) in enumerate(DIMS, 1):
            HC = H * C
            b2_t[li] = lay.tile([128, HC], bf16, tag=f"b2{li}", name=f"b2{li}t")
            nc.sync.dma_start(out=b2_t[li][:], in_=inp[f"b2{li}"][:])
        wsb_t = {}
        for li in (2, 3):
            fin, H, C = DIMS[li - 1]
            HC = H * C
            nkc = fin // 128
            wsb_t[li] = lay.tile([128, nkc * (HC + 8)], bf16, tag=f"wsb{li}", name=f"wsb{li}t")
            for kc in range(nkc):
                nc.sync.dma_start(
                    out=wsb_t[li][:, kc * (HC + 8):(kc + 1) * (HC + 8)],
                    in_=inp[f"Wext{li}"][kc * 128:(kc + 1) * 128, :])

        pool_sb = consts.tile([64, 257], f32)
        nc.any.memset(pool_sb[:], 0.0)

        def rf_rows(st):
            q, r = st // RFC, st % RFC
            return rf_d[q], r

        # copy host idx/batch cols into recfull chunks
        for q in range(4):
            r0 = q * RFC
            nr = min(RFC, NB * 128 - r0)
            nc.scalar.dma_start(out=rf_d[q][:, 0:2 * T + 1],
                                in_=inp["rec"][r0:r0 + nr, 0:2 * T + 1])

        # ---------------- phase A emitters ----------------
        def phaseA1(rt0, nb):
            xs = sb.tile([128, nb * 52], bf16, tag="xsA1", bufs=3)
            nc.sync.dma_start(
                out=xs[:].rearrange("p (j w) -> p j w", j=nb),
                in_=inp["xrow"][rt0 * 128:(rt0 + nb) * 128, :]
                    .rearrange("(j p) w -> p j w", p=128))
            xtt = sb.tile([32, nb * 128], bf16, tag="xtt", bufs=3)
            nc.scalar.dma_start(out=xtt[:],
                                in_=inp["xT"][:, rt0 * 128:(rt0 + nb) * 128])
            p8 = pst.tile([128, nb * 8], f32, tag="ps3")
            for j in range(nb):
                nc.tensor.matmul(p8[:, j * 8:(j + 1) * 8],
                                 xtt[:, j * 128:(j + 1) * 128], Wasd1[:],
                                 start=True, stop=True)
            nc.vector.tensor_copy(
                out=xs[:].rearrange("p (j w) -> p j w", j=nb)[:, :, 32:48]
                    .bitcast(f32),
                in_=p8[:].rearrange("p (j w) -> p j w", j=nb))
            nc.scalar.activation(
                out=xs[:].rearrange("p (j w) -> p j w", j=nb)[:, :, 48:52],
                in_=p8[:].rearrange("p (j w) -> p j w", j=nb)[:, :, 4:8],
                func=AF.Copy)
            h = rt0 * 128 // R2
            o = rt0 * 128 - h * R2
            nc.sync.dma_start(
                out=xe_d[0][h][o:o + nb * 128, :]
                    .rearrange("(j p) w -> p j w", p=128),
                in_=xs[:].rearrange("p (j w) -> p j w", j=nb))

        def phaseA(li, rt):
            fin, H, C = DIMS[li - 1]
            HC = H * C
            P = PW[li - 1]
            nkc = fin // 128
            wsb = wsb_t[li]
            pxs = psb.tile([128, 512], f32, tag="big")
            p8 = pst.tile([128, 8], f32, tag="ps3")
            hh_ = rt * 128 // R2
            ro = rt * 128 - hh_ * R2
            for kc in range(nkc):
                ht = sb.tile([128, 128], bf16, tag="ht", bufs=8)
                nc.sync.dma_start(
                    out=ht[:],
                    in_=hT_d[li - 1][hh_][ro:ro + 128,
                                          kc * 128:(kc + 1) * 128],
                    transpose=True)
                W0 = kc * (HC + 8)
                nc.tensor.matmul(pxs[:, 0:HC], ht[:], wsb[:, W0:W0 + HC],
                                 start=(kc == 0), stop=(kc == nkc - 1))
                nc.tensor.matmul(p8[:], ht[:], wsb[:, W0 + HC:W0 + HC + 8],
                                 start=(kc == 0), stop=(kc == nkc - 1))
            xs = sb.tile([128, P + 20], bf16, tag="xsA", bufs=4)
            nc.scalar.activation(out=xs[:, 0:P], in_=pxs[:, 0:P], func=AF.Copy)
            nc.scalar.activation(out=xs[:, P:P + 16].bitcast(f32), in_=p8[:],
                                 func=AF.Copy)
            nc.scalar.activation(out=xs[:, P + 16:P + 20], in_=p8[:, 4:8],
                                 func=AF.Copy)
            nc.scalar.dma_start(out=xe_d[li - 1][hh_][ro:ro + 128, :],
                                in_=xs[:])

        def ag(li, half):
            xf = xf_d[li - 1]
            nc.gpsimd.collective_compute(
                "AllGather", AO.bypass, replica_groups=RG,
                ins=[xe_d[li - 1][half][:]],
                outs=[xf[half * D * R2:(half + 1) * D * R2, :]])

        # ---------------- block emitter ----------------
        def block(li, i):
            fin, H, C = DIMS[li - 1]
            HC = H * C
            P = PW[li - 1]
            Wp = W2[li - 1]
            scheme_h = (li == 1)
            st128 = i * 128
            stblk = i * BLK
            xe = xe_d[li - 1]
            xf = xf_d[li - 1]
            b2 = b2_t[li]

            rft, ro_ = rf_rows(st128)
            if scheme_h:
                rfl = sb.tile([128, RECW], i32, tag="rfl", bufs=3)
                nc.sync.dma_start(out=rfl[:],
                                  in_=inp["rec"][st128:st128 + 128, :])
                eatt = sb2.tile([6, T * 128], f32, tag="eatt", bufs=3)
                nc.scalar.dma_start(
                    out=eatt[:],
                    in_=inp["eaT"][:, st128 * T:(st128 + 128) * T])
            else:
                rfl = sb.tile([128, RFW], i32, tag="rfl", bufs=3)
                nc.sync.dma_start(out=rfl[:], in_=rft[ro_:ro_ + 128, :])
                slt_t = sb.tile([128, 64 * T], i32, tag="sllt", bufs=3)
                nc.scalar.dma_start(
                    out=slt_t[:],
                    in_=inp["rec"][st128:st128 + 128,
                                   C_SLLT:C_SLLT + 64 * T])
            xsl = sb.tile([128, Wp], bf16, tag="xsl")
            if stblk + 128 <= R2:
                nc.scalar.dma_start(out=xsl[:], in_=xe[0][stblk:stblk + 128, :])
            elif stblk >= R2:
                nc.scalar.dma_start(out=xsl[:],
                                    in_=xe[1][stblk - R2:stblk - R2 + 128, :])
            else:
                nlo = R2 - stblk
                nc.scalar.dma_start(out=xsl[0:nlo, :], in_=xe[0][stblk:R2, :])
                nc.scalar.dma_start(out=xsl[nlo:128, :],
                                    in_=xe[1][0:128 - nlo, :])
            gat = sbg.tile([128, T * Wp], bf16, tag="gat", bufs=3)
            for t in range(T):
                nc.gpsimd.indirect_dma_start(
                    out=gat[:, t * Wp:(t + 1) * Wp], out_offset=None,
                    in_=xf[:],
                    in_offset=bass.IndirectOffsetOnAxis(ap=rfl[:, t:t + 1],
                                                        axis=0))
            if scheme_h:
                sllT = rfl[:, C_SLLT:C_SLLT + 64 * T].bitcast(bf16)
            else:
                sllT = slt_t[:].bitcast(bf16)
            atp = psb.tile([128, 4 * T], f32, tag="big")
            for t in range(T):
                nc.tensor.matmul(atp[:, t * 4:(t + 1) * 4],
                                 sllT[:, t * 128:(t + 1) * 128],
                                 xsl[:, P + 16:P + 20],
                                 start=True, stop=True)
            if scheme_h:
                rel = rfl[:, 2 * T + 1:3 * T + 1].bitcast(f32)
                sall_t = sbg.tile([128, T * 128], bf16, tag="salle", bufs=3)
                nc.vector.tensor_tensor(
                    out=sall_t[:].rearrange("p (t n) -> p t n", t=T),
                    in0=rel.unsqueeze(2).to_broadcast([128, T, 128]),
                    in1=io32[:].unsqueeze(1).to_broadcast([128, T, 128]),
                    op=AO.is_equal)
                sall = sall_t[:]
                nc.sync.dma_start(out=rft[ro_:ro_ + 128, C_SALL:C_ALE],
                                  in_=sall_t[:].bitcast(i32))
                # al_e for all 3 layers (edges) via PE from transposed ea
                alp = pss.tile([128, 12 * T], f32, tag="sm1")
                for t in range(T):
                    nc.tensor.matmul(alp[:, t * 12:(t + 1) * 12],
                                     eatt[:, t * 128:(t + 1) * 128],
                                     Aecat[:], start=True, stop=True)
                ale = sb.tile([128, 12 * T + 12], f32, tag="ale", bufs=3)
                a3 = ale[:].rearrange("p (l x) -> p l x", l=3)
                nc.vector.tensor_copy(
                    out=a3[:, :, 0:4 * T].rearrange("p l (t h) -> p l t h",
                                                    t=T),
                    in_=alp[:].rearrange("p (t l h) -> p l t h", l=3, h=4))
                alet = None
            else:
                sall = rfl[:, C_SALL:C_ALE].bitcast(bf16)
                alet = rfl[:, C_ALE + (li - 1) * (4 * T + 4):
                           C_ALE + li * (4 * T + 4)].bitcast(f32)
            # logits (fp32): al_s[src] + al_d[dst] + al_e
            alsw = sb.tile([128, T * 16], bf16, tag="alsw")
            nc.vector.tensor_copy(
                out=alsw[:].rearrange("p (t w) -> p t w", t=T),
                in_=gat[:].rearrange("p (t w) -> p t w", t=T)[:, :, P:P + 16])
            wall = sb.tile([128, T * 4], f32, tag="wall")
            nc.vector.tensor_tensor(
                out=wall[:].rearrange("p (t h) -> p t h", t=T),
                in0=alsw[:].bitcast(f32).rearrange(
                    "p (t w) -> p t w", t=T)[:, :, 0:4],
                in1=atp[:].rearrange("p (t h) -> p t h", t=T),
                op=AO.add)
            ale_e = a3[:, 0, 0:4 * T] if scheme_h else alet[:, 0:4 * T]
            nc.vector.tensor_tensor(out=wall[:], in0=wall[:],
                                    in1=ale_e, op=AO.add)
            lk = sb.tile([128, T * 4], f32, tag="lk")
            nc.vector.tensor_scalar(out=lk[:], in0=wall[:], scalar1=0.2,
                                    scalar2=None, op0=AO.mult)
            nc.vector.tensor_tensor(out=wall[:], in0=wall[:], in1=lk[:],
                                    op=AO.max)
            wallb = sb.tile([128, T * 4], bf16, tag="wallb")
            nc.scalar.activation(out=wallb[:], in_=wall[:], func=AF.Exp)
            # values: w * payload (+ w in trailing cols for denominator)
            if scheme_h:
                VW = 140
                val = sbg.tile([128, T * VW], bf16, tag="val")
                v3 = val[:].rearrange("p (t w) -> p t w", t=T)
                nc.vector.tensor_tensor(
                    out=v3[:, :, 0:128].rearrange("p t (h c) -> p t h c", h=4),
                    in0=gat[:].rearrange("p (t w) -> p t w", t=T)
                        [:, :, 0:32].unsqueeze(2).to_broadcast([128, T, 4, 32]),
                    in1=wallb[:].rearrange("p (t h) -> p t h", t=T)
                        .unsqueeze(3).to_broadcast([128, T, 4, 32]),
                    op=AO.mult)
                nc.vector.tensor_copy(
                    out=v3[:, :, 128:132],
                    in_=wallb[:].rearrange("p (t h) -> p t h", t=T))
                nc.vector.tensor_copy(
                    out=v3[:, :, 132:140],
                    in_=rfl[:, 3 * T + 1:7 * T + 1].bitcast(bf16)
                        .rearrange("p (t w) -> p t w", t=T))
                nps = pss.tile([128, VW], f32, tag="sm")
                for t in range(T):
                    nc.tensor.matmul(nps[:], sall[:, t * 128:(t + 1) * 128],
                                     val[:, t * VW:(t + 1) * VW],
                                     start=(t == 0), stop=(t == T - 1))
                # emean + self-loop / edge al_e epilogue rows (cols 132:140
                # hold one-hot-scattered [ea|deg] sums)
                degr = sb.tile([128, 1], f32, tag="degr")
                nc.vector.tensor_scalar(out=degr[:], in0=nps[:, 138:139],
                                        scalar1=1.0, scalar2=None, op0=AO.max)
                nc.vector.reciprocal(out=degr[:], in_=degr[:])
                em = sb.tile([128, 8], f32, tag="em")
                nc.vector.tensor_scalar(out=em[:, 0:6], in0=nps[:, 132:138],
                                        scalar1=degr[:], scalar2=None,
                                        op0=AO.mult)
                emt_ps = pst.tile([6, 128], f32, tag="ps3")
                nc.tensor.transpose(emt_ps[:], em[:, 0:6], ident[:])
                emt = sb.tile([6, 128], f32, tag="emts")
                nc.vector.tensor_copy(out=emt[:], in_=emt_ps[:])
                asps = pst.tile([128, 12], f32, tag="ps3")
                nc.tensor.matmul(asps[:], emt[:], Aecat[:],
                                 start=True, stop=True)
                nc.vector.tensor_copy(
                    out=a3[:, :, 4 * T:4 * T + 4],
                    in_=asps[:].rearrange("p (l h) -> p l h", l=3))
                nc.scalar.dma_start(out=rft[ro_:ro_ + 128, C_ALE:RFW],
                                    in_=ale[:].bitcast(i32))
            else:
                g3 = gat[:].rearrange("p (t w) -> p t w", t=T)
                nc.vector.tensor_tensor(
                    out=g3[:, :, 0:P].rearrange("p t (h c) -> p t h c", h=4),
                    in0=g3[:, :, 0:P].rearrange("p t (h c) -> p t h c", h=4),
                    in1=wallb[:].rearrange("p (t h) -> p t h", t=T)
                        .unsqueeze(3).to_broadcast([128, T, 4, C]),
                    op=AO.mult)
                nc.vector.tensor_copy(
                    out=g3[:, :, P:P + 4],
                    in_=wallb[:].rearrange("p (t h) -> p t h", t=T))
                if li == 2:
                    nps1 = pss.tile([128, 256], f32, tag="sm1")
                    nps = pss.tile([128, 260], f32, tag="sm")
                    for t in range(T):
                        nc.tensor.matmul(nps1[:],
                                         sall[:, t * 128:(t + 1) * 128],
                                         gat[:, t * Wp:t * Wp + 256],
                                         start=(t == 0), stop=(t == T - 1))
                        nc.tensor.matmul(nps[:],
                                         sall[:, t * 128:(t + 1) * 128],
                                         gat[:, t * Wp + 256:t * Wp + 516],
                                         start=(t == 0), stop=(t == T - 1))
                else:
                    nps = pss.tile([128, 260], f32, tag="sm")
                    for t in range(T):
                        nc.tensor.matmul(nps[:],
                                         sall[:, t * 128:(t + 1) * 128],
                                         gat[:, t * Wp:t * Wp + 260],
                                         start=(t == 0), stop=(t == T - 1))
            # self-loop weight
            u = xsl[:, P:P + 16].bitcast(f32)
            sl0 = sb.tile([128, 4], f32, tag="sl0")
            nc.vector.tensor_tensor(out=sl0[:], in0=u[:, 0:4], in1=u[:, 4:8],
                                    op=AO.add)
            ale_s = a3[:, 0, 4 * T:4 * T + 4] if scheme_h \
                else alet[:, 4 * T:4 * T + 4]
            nc.vector.tensor_tensor(out=sl0[:], in0=sl0[:],
                                    in1=ale_s, op=AO.add)
            lk2 = sb.tile([128, 4], f32, tag="lk2")
            nc.vector.tensor_scalar(out=lk2[:], in0=sl0[:], scalar1=0.2,
                                    scalar2=None, op0=AO.mult)
            nc.vector.tensor_tensor(out=sl0[:], in0=sl0[:], in1=lk2[:],
                                    op=AO.max)
            sl = sb.tile([128, 4], f32, tag="sl")
            nc.scalar.activation(out=sl[:], in_=sl0[:], func=AF.Exp)
            slb = sb.tile([128, 4], bf16, tag="slb")
            nc.vector.tensor_copy(out=slb[:], in_=sl[:])
            # denominator
            NPSc = 132 if scheme_h else 260
            den = sb.tile([128, 4], f32, tag="den")
            nc.vector.tensor_tensor(out=den[:], in0=nps[:, NPSc - 4:NPSc],
                                    in1=sl[:], op=AO.add)
            nc.vector.reciprocal(out=den[:], in_=den[:])
            denb = sb.tile([128, 4], bf16, tag="denb")
            nc.vector.tensor_copy(out=denb[:], in_=den[:])

            hh = sb2.tile([128, HC], bf16, tag="hh")
            if scheme_h:
                aggb = sb.tile([128, 128], bf16, tag="aggb")
                nc.scalar.activation(out=aggb[:], in_=nps[:, 0:128],
                                     func=AF.Copy)
                tmp = sb.tile([128, 128], bf16, tag="tmpL1")
                nc.vector.tensor_tensor(
                    out=tmp[:].rearrange("p (h c) -> p h c", h=4),
                    in0=xsl[:, 0:32].unsqueeze(1).to_broadcast([128, 4, 32]),
                    in1=slb[:].unsqueeze(2).to_broadcast([128, 4, 32]),
                    op=AO.mult)
                nc.vector.tensor_tensor(out=aggb[:], in0=aggb[:], in1=tmp[:],
                                        op=AO.add)
                nc.vector.tensor_tensor(
                    out=aggb[:].rearrange("p (h c) -> p h c", h=4),
                    in0=aggb[:].rearrange("p (h c) -> p h c", h=4),
                    in1=denb[:].unsqueeze(2).to_broadcast([128, 4, 32]),
                    op=AO.mult)
                agT_ps = pst.tile([128, 128], bf16, tag="ps3")
                nc.tensor.transpose(agT_ps[:], aggb[:], identb[:])
                agT = sb.tile([128, 128], bf16, tag="agT")
                nc.vector.tensor_copy(out=agT[:], in_=agT_ps[:])
                hps = psb.tile([128, 512], f32, tag="big")
                nc.tensor.matmul(hps[:, 0:256], agT[:], W1sb[:],
                                 start=True, stop=True)
                hpre = sb.tile([128, HC], bf16, tag="hpre")
                nc.scalar.activation(out=hpre[:], in_=hps[:, 0:256],
                                     func=AF.Copy)
            else:
                aggb = sb.tile([128, HC], bf16, tag="aggb")
                if li == 2:
                    nc.scalar.activation(out=aggb[:, 0:256], in_=nps1[:],
                                         func=AF.Copy)
                    nc.scalar.activation(out=aggb[:, 256:512],
                                         in_=nps[:, 0:256], func=AF.Copy)
                else:
                    nc.scalar.activation(out=aggb[:], in_=nps[:, 0:HC],
                                         func=AF.Copy)
                tmp = sb2.tile([128, HC], bf16, tag="tmpL")
                nc.vector.tensor_tensor(
                    out=tmp[:].rearrange("p (h c) -> p h c", h=4),
                    in0=xsl[:, 0:P].rearrange("p (h c) -> p h c", h=4),
                    in1=slb[:].unsqueeze(2).to_broadcast([128, 4, C]),
                    op=AO.mult)
                nc.vector.tensor_tensor(out=aggb[:], in0=aggb[:], in1=tmp[:],
                                        op=AO.add)
                hpre = aggb
                nc.vector.tensor_tensor(
                    out=hpre[:].rearrange("p (h c) -> p h c", h=4),
                    in0=hpre[:].rearrange("p (h c) -> p h c", h=4),
                    in1=denb[:].unsqueeze(2).to_broadcast([128, 4, C]),
                    op=AO.mult)
            # BN bias (scale folded into W on host) + ELU
            nc.vector.tensor_tensor(out=hpre[:], in0=hpre[:], in1=b2[:],
                                    op=AO.add)
            zn = sb2.tile([128, HC], bf16, tag="zn")
            nc.vector.tensor_scalar(out=zn[:], in0=hpre[:], scalar1=0.0,
                                    scalar2=None, op0=AO.min)
            nc.scalar.activation(out=zn[:], in_=zn[:], func=AF.Exp)
            rl = sb2.tile([128, HC], bf16, tag="rl")
            nc.vector.tensor_scalar(out=rl[:], in0=hpre[:], scalar1=0.0,
                                    scalar2=None, op0=AO.max)
            nc.vector.scalar_tensor_tensor(
                out=hh[:], in0=zn[:], scalar=-1.0, in1=rl[:],
                op0=AO.add, op1=AO.add)
            if li < 3:
                if stblk + BLK <= R2:
                    nc.sync.dma_start(out=hT_d[li][0][stblk:stblk + BLK, :],
                                      in_=hh[0:BLK, :])
                elif stblk >= R2:
                    nc.sync.dma_start(
                        out=hT_d[li][1][stblk - R2:stblk - R2 + BLK, :],
                        in_=hh[0:BLK, :])
                else:
                    nlo = R2 - stblk
                    nc.sync.dma_start(out=hT_d[li][0][stblk:R2, :],
                                      in_=hh[0:nlo, :])
                    nc.sync.dma_start(out=hT_d[li][1][0:BLK - nlo, :],
                                      in_=hh[nlo:BLK, :])
            else:
                bcol = rfl[:, 2 * T:2 * T + 1].bitcast(f32)
                bt = sb.tile([128, 64], bf16, tag="bt")
                nc.vector.tensor_tensor(out=bt[:],
                                        in0=bcol.to_broadcast([128, 64]),
                                        in1=io64[:], op=AO.is_equal)
                pps = pst.tile([64, 257], f32, tag="ps3")
                nc.tensor.matmul(pps[:, 0:HC], bt[:], hh[:],
                                 start=True, stop=True)
                nc.tensor.matmul(pps[:, HC:HC + 1], bt[:], onescolb[:],
                                 start=True, stop=True)
                nc.vector.tensor_tensor(out=pool_sb[:], in0=pool_sb[:],
                                        in1=pps[:], op=AO.add)

        # ---------------- program ----------------
        for rt0 in range(0, 25, 5):
            phaseA1(rt0, 5)
        ag(1, 0)
        for rt0 in range(25, NB, 5):
            phaseA1(rt0, 5)
        ag(1, 1)

        # ---- block loops; next-layer linear tiles emitted in two batches ----
        # (in-order engine queues: emit work only once its deps are ~ready)
        for li in (1, 2, 3):
            for i in range(NB):
                block(li, i)
                if li < 3:
                    if i == 25:          # hT lo-half complete after block 25
                        for rt in range(0, 25):
                            phaseA(li + 1, rt)
                    if i == 41:
                        ag(li + 1, 0)
                    if i == NB - 1:      # hi-half hT complete
                        for rt in range(25, NB):
                            phaseA(li + 1, rt)
                        ag(li + 1, 1)

        # ---------------- final MLP ----------------
        nc.sync.dma_start(out=pool_i[:], in_=pool_sb[:])
        nc.gpsimd.collective_compute("AllReduce", AO.add, replica_groups=RG,
                                     ins=[pool_i[:]], outs=[pool_o[:]])
        pool2 = sb.tile([64, 257], f32, tag="pool2")
        nc.sync.dma_start(out=pool2[:], in_=pool_o[:])
        cnt = sb.tile([64, 1], f32, tag="cnt")
        nc.vector.tensor_scalar(out=cnt[:], in0=pool2[:, 256:257], scalar1=1.0,
                                scalar2=None, op0=AO.max)
        nc.vector.reciprocal(out=cnt[:], in_=cnt[:])
        nc.vector.tensor_scalar(out=pool2[:, 0:256], in0=pool2[:, 0:256],
                                scalar1=cnt[:], scalar2=None, op0=AO.mult)
        pts = sb.tile([128, 128], f32, tag="pts")
        for ch in range(2):
            ptp = pst.tile([128, 64], f32, tag="ps3")
            nc.tensor.transpose(ptp[:], pool2[:, ch * 128:(ch + 1) * 128],
                                ident[0:64, 0:64])
            nc.vector.tensor_copy(out=pts[:, ch * 64:(ch + 1) * 64],
                                  in_=ptp[:])
        wf1 = sb.tile([128, 64], f32, tag="wf1")
        for ch in range(2):
            nc.sync.dma_start(out=wf1[:, ch * 32:(ch + 1) * 32],
                              in_=inp["Wf1"][ch * 128:(ch + 1) * 128, :])
        z1p = pst.tile([64, 32], f32, tag="ps3")
        for ch in range(2):
            nc.tensor.matmul(z1p[:], pts[:, ch * 64:(ch + 1) * 64],
                             wf1[:, ch * 32:(ch + 1) * 32],
                             start=(ch == 0), stop=(ch == 1))
        gf = sb.tile([64, 32], f32, tag="gf")
        nc.sync.dma_start(out=gf[:], in_=inp["gfr"][:])
        nc.vector.tensor_scalar(out=gf[:], in0=gf[:], scalar1=BNC,
                                scalar2=None, op0=AO.mult)
        b2f = sb.tile([64, 32], f32, tag="b2f")
        nc.sync.dma_start(out=b2f[:], in_=inp["bf1r"][:])
        nc.vector.tensor_tensor(out=b2f[:], in0=b2f[:], in1=gf[:], op=AO.mult)
        bbf = sb.tile([64, 32], f32, tag="bbf")
        nc.sync.dma_start(out=bbf[:], in_=inp["bbfr"][:])
        nc.vector.tensor_tensor(out=b2f[:], in0=b2f[:], in1=bbf[:], op=AO.add)
        zf = sb.tile([64, 32], f32, tag="zf")
        nc.vector.tensor_tensor(out=zf[:], in0=z1p[:], in1=gf[:], op=AO.mult)
        nc.vector.tensor_tensor(out=zf[:], in0=zf[:], in1=b2f[:], op=AO.add)
        zn2 = sb.tile([64, 32], f32, tag="zn2")
        nc.vector.tensor_scalar(out=zn2[:], in0=zf[:], scalar1=0.0,
                                scalar2=None, op0=AO.min)
        nc.scalar.activation(out=zn2[:], in_=zn2[:], func=AF.Exp)
        rl2 = sb.tile([64, 32], f32, tag="rl2")
        nc.vector.tensor_scalar(out=rl2[:], in0=zf[:], scalar1=0.0,
                                scalar2=None, op0=AO.max)
        nc.vector.scalar_tensor_tensor(out=zf[:], in0=zn2[:], scalar=-1.0,
                                       in1=rl2[:], op0=AO.add, op1=AO.add)
        ztp = pst.tile([32, 64], f32, tag="ps3")
        nc.tensor.transpose(ztp[:], zf[:], ident[0:64, 0:64])
        zts = sb.tile([32, 64], f32, tag="zts")
        nc.vector.tensor_copy(out=zts[:], in_=ztp[:])
        wf2 = sb.tile([32, 2], f32, tag="wf2")
        nc.sync.dma_start(out=wf2[:], in_=inp["Wf2"][:])
        z2p = pst.tile([64, 2], f32, tag="ps3")
        nc.tensor.matmul(z2p[:], zts[:], wf2[:], start=True, stop=True)
        bf2 = sb.tile([64, 2], f32, tag="bf2")
        nc.sync.dma_start(out=bf2[:], in_=inp["bf2r"][:])
        z2 = sb.tile([64, 2], f32, tag="z2")
        nc.vector.tensor_tensor(out=z2[:], in0=z2p[:], in1=bf2[:], op=AO.add)
        mrow = sb.tile([64, 1], f32, tag="mrow")
        nc.vector.tensor_reduce(out=mrow[:], in_=z2[:],
                                axis=mybir.AxisListType.X, op=AO.max)
        nc.vector.tensor_scalar(out=z2[:], in0=z2[:], scalar1=mrow[:],
                                scalar2=None, op0=AO.subtract)
        ez = sb.tile([64, 2], f32, tag="ez")
        nc.scalar.activation(out=ez[:], in_=z2[:], func=AF.Exp)
        ssum = sb.tile([64, 1], f32, tag="ssum")
        nc.vector.tensor_reduce(out=ssum[:], in_=ez[:],
                                axis=mybir.AxisListType.X, op=AO.add)
        nc.scalar.activation(out=ssum[:], in_=ssum[:], func=AF.Ln)
        nc.vector.tensor_scalar(out=z2[:], in0=z2[:], scalar1=ssum[:],
                                scalar2=None, op0=AO.subtract)
        nc.sync.dma_start(out=out_d[:, :], in_=z2[:])

    nc.compile()
    return nc


# ---------------------------------------------------------------- entry point
def kernel(**inputs):
    _patch_walrus()
    in_maps, T = _prep(inputs)
    if T not in _CACHE:
        _CACHE[T] = _build(T)
    nc = _CACHE[T]
    from concourse.bass_utils import run_bass_kernel_spmd
    res = run_bass_kernel_spmd(nc, in_maps, list(range(D))).results
    return np.asarray(res[0]["out"], dtype=np.float32)
